# revision 30
# baseline (speedup 1.0000x reference)
"""AdvancedRPN (conv3x3+relu -> cls/bbox heads -> decode -> top-1000 -> NMS)
on 8 Trainium2 NeuronCores.

Device: spatial bands of 25 rows per core run the fp32 conv3x3 + head
matmuls (the compute-dominant part; fp32 is precision-mandated: score
errors ~1e-7 flip top-k output rows vs the fp32 reference). Each core
returns its head maps; the tiny decode/top-k/NMS tail (<<1% of FLOPs)
runs on host in reference-exact fp32 numpy.
"""

import numpy as np

import concourse.bass as bass
import concourse.mybir as mybir
from concourse.bacc import Bacc
from concourse import tile as tile_mod
from concourse.bass_utils import run_bass_kernel_spmd

F32 = mybir.dt.float32

A = 3
FH = FW = 200
RB = 25              # output rows per core
W2 = 202             # padded row width
S = RB * W2          # 5050 spatial cols per core
FINR = RB + 2        # input rows incl halo
FS = FINR * W2 + 2   # flat input cols + halo-read pad
K_OUT = 1000
NMS_T = 0.7
MIN_SIZE = 1e-3
BBOX_XFORM_CLIP = float(np.log(1000.0 / 16.0))
NCORES = 8
ACTF = mybir.ActivationFunctionType


def build_module():
    nc = Bacc(trn_type="TRN2", num_devices=NCORES)

    feat = nc.dram_tensor("feat", [2, 128, FS], F32, kind="ExternalInput")
    wconv = nc.dram_tensor("wconv", [128, 36 * 128], F32, kind="ExternalInput")
    whead = nc.dram_tensor("whead", [128, 30], F32, kind="ExternalInput")
    heads_t = nc.dram_tensor("heads_out", [15, S], F32, kind="ExternalOutput")

    with tile_mod.TileContext(nc) as tc:
        with (
            tc.tile_pool(name="ps", bufs=1, space="PSUM") as ppool,
            tc.tile_pool(name="sb", bufs=1) as sb,
        ):
            wc_sb = sb.tile([128, 36 * 128], F32, tag="wc")
            wh_sb = sb.tile([128, 30], F32, tag="wh")
            feat_sb = [sb.tile([128, FS], F32, tag=f"feat{c}", name=f"feat_sb{c}")
                       for c in range(2)]
            t_sb = [sb.tile([128, S], F32, tag=f"t{c}", name=f"t_sb{c}")
                    for c in range(2)]
            heads = sb.tile([15, S], F32, tag="heads")

            nc.sync.dma_start(wc_sb[:, 0:18 * 128], wconv[:, 0:18 * 128])
            nc.sync.dma_start(wc_sb[:, 18 * 128:], wconv[:, 18 * 128:])
            nc.sync.dma_start(wh_sb[:], whead[:])
            # feature load in column chunks so the first conv tiles can start
            # while the rest of the band is still in flight
            NCH = 4
            chw = (FS + NCH - 1) // NCH
            for c in range(2):
                for ch in range(NCH):
                    c0 = ch * chw
                    c1 = min(FS, c0 + chw)
                    nc.sync.dma_start(feat_sb[c][:, c0:c1], feat[c][:, c0:c1])

            NT = 512
            ntiles = (S + NT - 1) // NT
            # j-outer so each spatial tile's head matmuls interleave with the
            # next tile's conv instead of serializing after the whole conv
            for j in range(ntiles):
                n0 = j * NT
                n1 = min(S, n0 + NT)
                n = n1 - n0
                for cot in range(2):
                    ps = ppool.tile([128, NT], F32, tag="cps", bufs=4)
                    first = True
                    for tap in range(9):
                        dy, dx = tap // 3, tap % 3
                        off = dy * W2 + dx
                        for cit in range(2):
                            blk = cot * 18 + tap * 2 + cit
                            nc.tensor.matmul(
                                ps[:, :n],
                                wc_sb[:, blk * 128:(blk + 1) * 128],
                                feat_sb[cit][:, off + n0: off + n0 + n],
                                start=first,
                                stop=(tap == 8 and cit == 1),
                            )
                            first = False
                    nc.scalar.activation(t_sb[cot][:, n0:n1], ps[:, :n], ACTF.Relu)
                ph = ppool.tile([15, NT], F32, tag="hps", bufs=2)
                for cit in range(2):
                    nc.tensor.matmul(
                        ph[:, :n],
                        wh_sb[:, cit * 15:(cit + 1) * 15],
                        t_sb[cit][:, n0:n1],
                        start=(cit == 0), stop=(cit == 1),
                    )
                nc.scalar.activation(heads[:, n0:n1], ph[:, :n], ACTF.Copy)
                nc.sync.dma_start(heads_t[:, n0:n1], heads[:, n0:n1])

    nc.compile()
    return nc


_CACHE = {}


def _host_inputs(features, conv_w, cls_w, bbox_w):
    f = np.asarray(features, np.float32)[0]
    wc = np.asarray(conv_w, np.float32)
    clw = np.asarray(cls_w, np.float32)[:, :, 0, 0]
    bww = np.asarray(bbox_w, np.float32)[:, :, 0, 0]

    fpad = np.zeros((256, FH + 2, W2), np.float32)
    fpad[:, 1:FH + 1, 1:FW + 1] = f

    wconv = np.empty((128, 36, 128), np.float32)
    for tap in range(9):
        dy, dx = tap // 3, tap % 3
        for cit in range(2):
            for cot in range(2):
                blk = cot * 18 + tap * 2 + cit
                wconv[:, blk, :] = wc[cot * 128:(cot + 1) * 128,
                                      cit * 128:(cit + 1) * 128, dy, dx].T
    wconv = wconv.reshape(128, 36 * 128)

    # head rows: class*3 + a; class 0 = score, 1..4 = dx,dy,dw,dh
    wh = np.zeros((128, 2, 15), np.float32)
    for cit in range(2):
        sl = slice(cit * 128, (cit + 1) * 128)
        for a in range(A):
            wh[:, cit, 0 * 3 + a] = clw[a, sl]
            for comp in range(4):
                wh[:, cit, (comp + 1) * 3 + a] = bww[a * 4 + comp, sl]
    wh = wh.reshape(128, 30)

    in_maps = []
    for r in range(NCORES):
        y0 = r * RB
        feat_r = np.zeros((256, FS), np.float32)
        feat_r[:, :FINR * W2] = fpad[:, y0:y0 + FINR, :].reshape(256, FINR * W2)
        in_maps.append({
            "feat": np.stack([feat_r[:128], feat_r[128:]], axis=0).copy(),
            "wconv": wconv,
            "whead": wh,
        })
    return in_maps


def _host_tail(heads_all, image_h, image_w):
    """Reference-exact fp32 decode / top-k / NMS from device head maps."""
    sc = np.zeros((FH, FW, A), np.float32)
    dl = np.zeros((FH, FW, A, 4), np.float32)
    for r in range(NCORES):
        h = heads_all[r].reshape(15, RB, W2)[:, :, 0:FW]   # [15, 25, 200]
        y0 = r * RB
        for a in range(A):
            sc[y0:y0 + RB, :, a] = h[a]
            for comp in range(4):
                dl[y0:y0 + RB, :, a, comp] = h[(comp + 1) * 3 + a]
    scores_all = sc.reshape(-1)
    deltas = dl.reshape(-1, 4)

    scales = np.array([32.0], np.float32)
    ratios = np.array([0.5, 1.0, 2.0], np.float32)
    h_r = np.sqrt(ratios)
    w_r = 1.0 / h_r
    ws = (w_r[:, None] * scales[None, :]).reshape(-1)
    hs = (h_r[:, None] * scales[None, :]).reshape(-1)
    cell = np.round(np.stack([-ws, -hs, ws, hs], axis=1) / 2.0).astype(np.float32)
    stride_h = np.float32(image_h // FH)
    stride_w = np.float32(image_w // FW)
    sx = np.arange(FW, dtype=np.float32) * stride_w
    sy = np.arange(FH, dtype=np.float32) * stride_h
    yy, xx = np.meshgrid(sy, sx, indexing="ij")
    shifts = np.stack([xx.ravel(), yy.ravel(), xx.ravel(), yy.ravel()], axis=1)
    anchors = (shifts[:, None, :] + cell[None, :, :]).reshape(-1, 4).astype(np.float32)

    wa = anchors[:, 2] - anchors[:, 0]
    ha = anchors[:, 3] - anchors[:, 1]
    cx = anchors[:, 0] + np.float32(0.5) * wa
    cy = anchors[:, 1] + np.float32(0.5) * ha
    dx, dy = deltas[:, 0], deltas[:, 1]
    dw = np.minimum(deltas[:, 2], np.float32(BBOX_XFORM_CLIP))
    dh = np.minimum(deltas[:, 3], np.float32(BBOX_XFORM_CLIP))
    pcx = dx * wa + cx
    pcy = dy * ha + cy
    pw = np.exp(dw) * wa
    ph = np.exp(dh) * ha
    boxes = np.stack([pcx - np.float32(0.5) * pw, pcy - np.float32(0.5) * ph,
                      pcx + np.float32(0.5) * pw, pcy + np.float32(0.5) * ph],
                     axis=1).astype(np.float32)

    idx = np.argsort(-scores_all, kind="stable")[:K_OUT]
    scores = scores_all[idx]
    b = boxes[idx]
    b = np.stack([np.clip(b[:, 0], 0, np.float32(image_w)),
                  np.clip(b[:, 1], 0, np.float32(image_h)),
                  np.clip(b[:, 2], 0, np.float32(image_w)),
                  np.clip(b[:, 3], 0, np.float32(image_h))], axis=1).astype(np.float32)
    valid = ((b[:, 2] - b[:, 0]) >= MIN_SIZE) & ((b[:, 3] - b[:, 1]) >= MIN_SIZE)

    area = (b[:, 2] - b[:, 0]) * (b[:, 3] - b[:, 1])
    xl = np.maximum(b[:, None, 0], b[None, :, 0])
    yt = np.maximum(b[:, None, 1], b[None, :, 1])
    xr = np.minimum(b[:, None, 2], b[None, :, 2])
    yb = np.minimum(b[:, None, 3], b[None, :, 3])
    inter = np.clip(xr - xl, 0, None) * np.clip(yb - yt, 0, None)
    union = area[:, None] + area[None, :] - inter
    iou = inter / (union + np.float32(1e-8))

    keep = valid.copy()
    sup = iou > NMS_T
    ar = np.arange(K_OUT)
    for i in range(K_OUT):
        if keep[i]:
            keep &= ~(sup[i] & (ar > i))
    out = np.where(keep[:, None],
                   np.concatenate([b, scores[:, None]], axis=1).astype(np.float32),
                   np.float32(0.0))
    return out, keep


def kernel(features, conv_w, conv_b, cls_w, cls_b, bbox_w, bbox_b,
           image_h, image_w):
    image_h = int(image_h)
    image_w = int(image_w)
    cb = np.asarray(conv_b, np.float32)
    clb = np.asarray(cls_b, np.float32)
    bwb = np.asarray(bbox_b, np.float32)
    assert np.all(cb == 0) and np.all(clb == 0) and np.all(bwb == 0), \
        "nonzero biases not supported by this kernel build"
    if "m" not in _CACHE:
        _CACHE["m"] = build_module()
    nc = _CACHE["m"]
    in_maps = _host_inputs(features, conv_w, cls_w, bbox_w)
    res = run_bass_kernel_spmd(nc, in_maps, core_ids=list(range(NCORES)))
    heads_all = [res.results[r]["heads_out"] for r in range(NCORES)]
    return _host_tail(heads_all, image_h, image_w)


# revision 32
# speedup vs baseline: 1.0010x; 1.0010x over previous
"""AdvancedRPN (conv3x3+relu -> cls/bbox heads -> decode -> top-1000 -> NMS)
on 8 Trainium2 NeuronCores.

Device: spatial bands of 25 rows per core run the fp32 conv3x3 + head
matmuls (the compute-dominant part; fp32 is precision-mandated: score
errors ~1e-7 flip top-k output rows vs the fp32 reference). Each core
returns its head maps; the tiny decode/top-k/NMS tail (<<1% of FLOPs)
runs on host in reference-exact fp32 numpy.
"""

import numpy as np

import concourse.bass as bass
import concourse.mybir as mybir
from concourse.bacc import Bacc
from concourse import tile as tile_mod
from concourse.bass_utils import run_bass_kernel_spmd

F32 = mybir.dt.float32

A = 3
FH = FW = 200
RB = 25              # output rows per core
W2 = 202             # padded row width
S = RB * W2          # 5050 spatial cols per core
FINR = RB + 2        # input rows incl halo
FS = FINR * W2 + 2   # flat input cols + halo-read pad
K_OUT = 1000
NMS_T = 0.7
MIN_SIZE = 1e-3
BBOX_XFORM_CLIP = float(np.log(1000.0 / 16.0))
NCORES = 8
ACTF = mybir.ActivationFunctionType


def build_module():
    nc = Bacc(trn_type="TRN2", num_devices=NCORES)

    feat = nc.dram_tensor("feat", [2, 128, FS], F32, kind="ExternalInput")
    wconv = nc.dram_tensor("wconv", [128, 36 * 128], F32, kind="ExternalInput")
    cbias = nc.dram_tensor("cbias", [2, 128, 1], F32, kind="ExternalInput")
    whead = nc.dram_tensor("whead", [128, 30], F32, kind="ExternalInput")
    heads_t = nc.dram_tensor("heads_out", [15, S], F32, kind="ExternalOutput")

    with tile_mod.TileContext(nc) as tc:
        with (
            tc.tile_pool(name="ps", bufs=1, space="PSUM") as ppool,
            tc.tile_pool(name="sb", bufs=1) as sb,
        ):
            wc_sb = sb.tile([128, 36 * 128], F32, tag="wc")
            cb_sb = [sb.tile([128, 1], F32, tag=f"cb{c}", name=f"cb_sb{c}")
                     for c in range(2)]
            wh_sb = sb.tile([128, 30], F32, tag="wh")
            feat_sb = [sb.tile([128, FS], F32, tag=f"feat{c}", name=f"feat_sb{c}")
                       for c in range(2)]
            t_sb = [sb.tile([128, S], F32, tag=f"t{c}", name=f"t_sb{c}")
                    for c in range(2)]
            heads = sb.tile([15, S], F32, tag="heads")

            nc.sync.dma_start(wc_sb[:, 0:18 * 128], wconv[:, 0:18 * 128])
            nc.sync.dma_start(wc_sb[:, 18 * 128:], wconv[:, 18 * 128:])
            nc.sync.dma_start(wh_sb[:], whead[:])
            for c in range(2):
                nc.sync.dma_start(cb_sb[c][:], cbias[c])
            # feature load in column chunks so the first conv tiles can start
            # while the rest of the band is still in flight
            NCH = 4
            chw = (FS + NCH - 1) // NCH
            for c in range(2):
                for ch in range(NCH):
                    c0 = ch * chw
                    c1 = min(FS, c0 + chw)
                    nc.sync.dma_start(feat_sb[c][:, c0:c1], feat[c][:, c0:c1])

            NT = 512
            ntiles = (S + NT - 1) // NT
            # j-outer so each spatial tile's head matmuls interleave with the
            # next tile's conv instead of serializing after the whole conv
            for j in range(ntiles):
                n0 = j * NT
                n1 = min(S, n0 + NT)
                n = n1 - n0
                for cot in range(2):
                    ps = ppool.tile([128, NT], F32, tag="cps", bufs=4)
                    first = True
                    for tap in range(9):
                        dy, dx = tap // 3, tap % 3
                        off = dy * W2 + dx
                        for cit in range(2):
                            blk = cot * 18 + tap * 2 + cit
                            nc.tensor.matmul(
                                ps[:, :n],
                                wc_sb[:, blk * 128:(blk + 1) * 128],
                                feat_sb[cit][:, off + n0: off + n0 + n],
                                start=first,
                                stop=(tap == 8 and cit == 1),
                            )
                            first = False
                    nc.vector.tensor_scalar(out=t_sb[cot][:, n0:n1], in0=ps[:, :n],
                                            scalar1=cb_sb[cot][:, 0:1], scalar2=0.0,
                                            op0=mybir.AluOpType.add,
                                            op1=mybir.AluOpType.max)
                ph = ppool.tile([15, NT], F32, tag="hps", bufs=2)
                for cit in range(2):
                    nc.tensor.matmul(
                        ph[:, :n],
                        wh_sb[:, cit * 15:(cit + 1) * 15],
                        t_sb[cit][:, n0:n1],
                        start=(cit == 0), stop=(cit == 1),
                    )
                nc.scalar.activation(heads[:, n0:n1], ph[:, :n], ACTF.Copy)
                nc.sync.dma_start(heads_t[:, n0:n1], heads[:, n0:n1])

    nc.compile()
    return nc


_CACHE = {}


def _host_inputs(features, conv_w, conv_b, cls_w, bbox_w):
    f = np.asarray(features, np.float32)[0]
    wc = np.asarray(conv_w, np.float32)
    cb = np.asarray(conv_b, np.float32).reshape(2, 128, 1)
    clw = np.asarray(cls_w, np.float32)[:, :, 0, 0]
    bww = np.asarray(bbox_w, np.float32)[:, :, 0, 0]

    fpad = np.zeros((256, FH + 2, W2), np.float32)
    fpad[:, 1:FH + 1, 1:FW + 1] = f

    wconv = np.empty((128, 36, 128), np.float32)
    for tap in range(9):
        dy, dx = tap // 3, tap % 3
        for cit in range(2):
            for cot in range(2):
                blk = cot * 18 + tap * 2 + cit
                wconv[:, blk, :] = wc[cot * 128:(cot + 1) * 128,
                                      cit * 128:(cit + 1) * 128, dy, dx].T
    wconv = wconv.reshape(128, 36 * 128)

    # head rows: class*3 + a; class 0 = score, 1..4 = dx,dy,dw,dh
    wh = np.zeros((128, 2, 15), np.float32)
    for cit in range(2):
        sl = slice(cit * 128, (cit + 1) * 128)
        for a in range(A):
            wh[:, cit, 0 * 3 + a] = clw[a, sl]
            for comp in range(4):
                wh[:, cit, (comp + 1) * 3 + a] = bww[a * 4 + comp, sl]
    wh = wh.reshape(128, 30)

    in_maps = []
    for r in range(NCORES):
        y0 = r * RB
        feat_r = np.zeros((256, FS), np.float32)
        feat_r[:, :FINR * W2] = fpad[:, y0:y0 + FINR, :].reshape(256, FINR * W2)
        in_maps.append({
            "feat": np.stack([feat_r[:128], feat_r[128:]], axis=0).copy(),
            "cbias": cb,
            "wconv": wconv,
            "whead": wh,
        })
    return in_maps


def _host_tail(heads_all, cls_b, bbox_b, image_h, image_w):
    """Reference-exact fp32 decode / top-k / NMS from device head maps."""
    clb = np.asarray(cls_b, np.float32)
    bwb = np.asarray(bbox_b, np.float32)
    sc = np.zeros((FH, FW, A), np.float32)
    dl = np.zeros((FH, FW, A, 4), np.float32)
    for r in range(NCORES):
        h = heads_all[r].reshape(15, RB, W2)[:, :, 0:FW]   # [15, 25, 200]
        y0 = r * RB
        for a in range(A):
            sc[y0:y0 + RB, :, a] = h[a] + clb[a]
            for comp in range(4):
                dl[y0:y0 + RB, :, a, comp] = h[(comp + 1) * 3 + a] + bwb[a * 4 + comp]
    scores_all = sc.reshape(-1)
    deltas = dl.reshape(-1, 4)

    scales = np.array([32.0], np.float32)
    ratios = np.array([0.5, 1.0, 2.0], np.float32)
    h_r = np.sqrt(ratios)
    w_r = 1.0 / h_r
    ws = (w_r[:, None] * scales[None, :]).reshape(-1)
    hs = (h_r[:, None] * scales[None, :]).reshape(-1)
    cell = np.round(np.stack([-ws, -hs, ws, hs], axis=1) / 2.0).astype(np.float32)
    stride_h = np.float32(image_h // FH)
    stride_w = np.float32(image_w // FW)
    sx = np.arange(FW, dtype=np.float32) * stride_w
    sy = np.arange(FH, dtype=np.float32) * stride_h
    yy, xx = np.meshgrid(sy, sx, indexing="ij")
    shifts = np.stack([xx.ravel(), yy.ravel(), xx.ravel(), yy.ravel()], axis=1)
    anchors = (shifts[:, None, :] + cell[None, :, :]).reshape(-1, 4).astype(np.float32)

    wa = anchors[:, 2] - anchors[:, 0]
    ha = anchors[:, 3] - anchors[:, 1]
    cx = anchors[:, 0] + np.float32(0.5) * wa
    cy = anchors[:, 1] + np.float32(0.5) * ha
    dx, dy = deltas[:, 0], deltas[:, 1]
    dw = np.minimum(deltas[:, 2], np.float32(BBOX_XFORM_CLIP))
    dh = np.minimum(deltas[:, 3], np.float32(BBOX_XFORM_CLIP))
    pcx = dx * wa + cx
    pcy = dy * ha + cy
    pw = np.exp(dw) * wa
    ph = np.exp(dh) * ha
    boxes = np.stack([pcx - np.float32(0.5) * pw, pcy - np.float32(0.5) * ph,
                      pcx + np.float32(0.5) * pw, pcy + np.float32(0.5) * ph],
                     axis=1).astype(np.float32)

    idx = np.argsort(-scores_all, kind="stable")[:K_OUT]
    scores = scores_all[idx]
    b = boxes[idx]
    b = np.stack([np.clip(b[:, 0], 0, np.float32(image_w)),
                  np.clip(b[:, 1], 0, np.float32(image_h)),
                  np.clip(b[:, 2], 0, np.float32(image_w)),
                  np.clip(b[:, 3], 0, np.float32(image_h))], axis=1).astype(np.float32)
    valid = ((b[:, 2] - b[:, 0]) >= MIN_SIZE) & ((b[:, 3] - b[:, 1]) >= MIN_SIZE)

    area = (b[:, 2] - b[:, 0]) * (b[:, 3] - b[:, 1])
    xl = np.maximum(b[:, None, 0], b[None, :, 0])
    yt = np.maximum(b[:, None, 1], b[None, :, 1])
    xr = np.minimum(b[:, None, 2], b[None, :, 2])
    yb = np.minimum(b[:, None, 3], b[None, :, 3])
    inter = np.clip(xr - xl, 0, None) * np.clip(yb - yt, 0, None)
    union = area[:, None] + area[None, :] - inter
    iou = inter / (union + np.float32(1e-8))

    keep = valid.copy()
    sup = iou > NMS_T
    ar = np.arange(K_OUT)
    for i in range(K_OUT):
        if keep[i]:
            keep &= ~(sup[i] & (ar > i))
    out = np.where(keep[:, None],
                   np.concatenate([b, scores[:, None]], axis=1).astype(np.float32),
                   np.float32(0.0))
    return out, keep


def kernel(features, conv_w, conv_b, cls_w, cls_b, bbox_w, bbox_b,
           image_h, image_w):
    image_h = int(image_h)
    image_w = int(image_w)
    if "m" not in _CACHE:
        _CACHE["m"] = build_module()
    nc = _CACHE["m"]
    in_maps = _host_inputs(features, conv_w, conv_b, cls_w, bbox_w)
    res = run_bass_kernel_spmd(nc, in_maps, core_ids=list(range(NCORES)))
    heads_all = [res.results[r]["heads_out"] for r in range(NCORES)]
    return _host_tail(heads_all, cls_b, bbox_b, image_h, image_w)
